# revision 17
# baseline (speedup 1.0000x reference)
"""Distributed causal multi-head attention for 8 TRN2 NeuronCores.

Problem: y = (softmax(mask(Q K^T / sqrt(d))) V) @ c_proj_w + c_proj_b with
Q,K,V = split(x @ c_attn_w + c_attn_b), shapes B=2, S=2048, NX=1024, NH=16,
HD=64.

Sharding: core c = (b, g) with b = c // 4, g = c % 4 — data parallel over the
batch, tensor parallel over 4 head-groups of 4 heads. Each core:
  1. computes qT/kT ([d, s] layout) and v ([s, d] layout) for its 4 heads from
     a host-pretransposed x[b]^T, so no on-device transposes are ever needed;
  2. runs causal attention in the "S^T" orientation: scores come out of the PE
     as [j, i] tiles, exp() is fused into the PSUM->SBUF copy on the scalar
     engine (no max-subtraction — scores are bounded), and the softmax
     denominator falls out of the PV matmul for free via a ones-column
     appended to V;
  3. AllGathers aT = (attention output)^T across its 4-core group and computes
     a 256-wide column slice of the output projection.
The host wrapper only slices/transposes inputs and concatenates outputs.
"""

import ml_dtypes
import numpy as np

import concourse.bass as bass
import concourse.mybir as mybir
from concourse import bacc, tile
from concourse.bass_utils import run_bass_kernel_spmd

B, S, NX, NH, HD = 2, 2048, 1024, 16, 64
NG = 4              # head-groups == cores per batch entry
HG = NH // NG       # heads per core
FG = HG * HD        # local feature width (256)
P = 128
SC = 512            # sequence chunk width
NSC = S // SC       # 4 chunks
KO = NX // P        # 8 contraction tiles
N_CORES = 8

F32 = mybir.dt.float32

# Compute dtype for PE matmuls. bf16 runs the PE at full rate with
# hardware fast-weight-load; fp32r streams rows at full rate but pays a
# ~512-cycle self-contained weight load per matmul (LDW cannot be split
# or cached for fp32/fp32r), an intrinsic ~2x overhead at N=512.
MM_DT = mybir.dt.bfloat16
F32R = mybir.dt.float32r

REPLICA_GROUPS = [[0, 1, 2, 3], [4, 5, 6, 7]]


def _mm(ap):
    """Matmul operands are already typed as MM_DT."""
    return ap


def build(nc: bass.Bass):
    xT = nc.declare_dram_parameter("xT", [NX, S], MM_DT, isOutput=False)
    wq = nc.declare_dram_parameter("wq", [NX, FG], MM_DT, isOutput=False)
    wk = nc.declare_dram_parameter("wk", [NX, FG], MM_DT, isOutput=False)
    wv = nc.declare_dram_parameter("wv", [NX, FG], MM_DT, isOutput=False)
    bqk = nc.declare_dram_parameter("bqk", [P, 4], F32, isOutput=False)
    bv = nc.declare_dram_parameter("bv", [P, FG], MM_DT, isOutput=False)
    wp = nc.declare_dram_parameter("wp", [NX, FG], MM_DT, isOutput=False)
    bp = nc.declare_dram_parameter("bp", [P, FG], F32, isOutput=False)
    maskw = nc.declare_dram_parameter("maskw", [P, 896], MM_DT, isOutput=False)
    onesd = nc.declare_dram_parameter("onesd", [1, P], F32R, isOutput=False)
    out = nc.declare_dram_parameter("out", [S, FG], F32, isOutput=True)

    # Per-chunk collective bounce buffers (collectives can't touch kernel I/O).
    ag_warm_in = nc.dram_tensor("ag_warm_in", [4, 128], MM_DT)
    ag_warm_out = nc.dram_tensor("ag_warm_out", [16, 128], MM_DT)
    aT_loc = [nc.dram_tensor(f"aT_loc{c}", [FG, SC], MM_DT) for c in range(NSC)]
    aT_full = [
        nc.dram_tensor(f"aT_full{c}", [NG * FG, SC], MM_DT) for c in range(NSC)
    ]

    with tile.TileContext(nc) as tc:
        nc_lp = nc.allow_low_precision(reason="float32r PE compute path")
        nc_lp.__enter__()
        with (
            tc.tile_pool(name="consts", bufs=1) as consts,
            tc.tile_pool(name="persist", bufs=1) as persist,
            tc.tile_pool(name="xt", bufs=2) as xt_pool,
            tc.tile_pool(name="pt", bufs=8) as pt_pool,
            tc.tile_pool(name="aTf", bufs=2) as aTf_pool,
            tc.tile_pool(name="outs", bufs=3) as out_pool,
            tc.tile_pool(name="small", bufs=4) as small,
            tc.tile_pool(name="psum", bufs=2, space="PSUM") as psum,
        ):
            # ---- load weights / constants ----
            wq_sb = consts.tile([P, KO, FG], MM_DT)
            wk_sb = consts.tile([P, KO, FG], MM_DT)
            wv_sb = consts.tile([P, KO, FG], MM_DT)
            wp_sb = consts.tile([P, KO, FG], MM_DT)
            bqk_sb = consts.tile([P, 4], F32)
            bv_sb = consts.tile([P, FG], MM_DT)
            bp_sb = consts.tile([P, FG], F32)
            maskw_sb = consts.tile([P, 896], MM_DT)
            ones128 = consts.tile([1, P], F32R)
            nc.sync.dma_start(wq_sb[:], wq.rearrange("(ko p) f -> p ko f", p=P))
            nc.sync.dma_start(bqk_sb[:], bqk[:])
            nc.sync.dma_start(wk_sb[:], wk.rearrange("(ko p) f -> p ko f", p=P))
            nc.sync.dma_start(wv_sb[:], wv.rearrange("(ko p) f -> p ko f", p=P))
            nc.sync.dma_start(bv_sb[:], bv[:])
            nc.gpsimd.dma_start(maskw_sb[:], maskw[:])
            nc.gpsimd.dma_start(ones128[:], onesd[:])
            nc.gpsimd.dma_start(wp_sb[:], wp.rearrange("(ko p) f -> p ko f", p=P))
            nc.gpsimd.dma_start(bp_sb[:], bp[:])

            # ---- persistent activation tiles ----
            # kT: [d, s] packed — tile hh holds heads (2hh, 2hh+1) on
            # partition halves; it is the scores lhsT ([128,128] weight
            # loads keep the PE fast-weight-load path).
            # qT: one zero-padded [128, s] tile per head, data on the same
            # partition half as in kT, zeros elsewhere — the zeros select
            # the head out of the packed kT during the scores matmul.
            # v: [s, d] per 128-row tile, a ones column at col 64 (softmax
            # denominator) and zero padding out to 128 columns so the PV
            # lhsT is a full [128,128] block.
            # aT: per-head [128, s]; only rows 0:64 are meaningful.
            qT_sb = [persist.tile([P, S], MM_DT, name=f"qT{h}") for h in range(HG)]
            kT_sb = [persist.tile([P, S], MM_DT, name=f"kT{hh}") for hh in range(2)]
            v_sb = [persist.tile([P, HG, P], MM_DT, name=f"v{st}") for st in range(S // P)]
            aT_sb = [persist.tile([P, S], MM_DT, name=f"aT{h}") for h in range(HG)]
            for h in range(HG):
                pad0 = (1 - h % 2) * HD
                nc.vector.memset(qT_sb[h][pad0:pad0 + HD, :], 0.0)

            nc.gpsimd.collective_compute(
                "AllGather",
                mybir.AluOpType.bypass,
                ins=[ag_warm_in[:].opt()],
                outs=[ag_warm_out[:].opt()],
                replica_groups=REPLICA_GROUPS,
            )

            # ===== per-chunk pipeline: QKV -> attention -> AllGather -> proj
            # Ascending order: attention for chunk sc only needs K/V of
            # chunks <= sc, so QKV(sc+1) overlaps attention(sc) and the
            # per-chunk AllGathers spread across the whole kernel.
            for sc in range(NSC):
                # ---- QKV for this chunk ----
                xt = xt_pool.tile([P, KO, SC], MM_DT, tag="xt")
                nc.sync.dma_start(
                    xt[:], xT.rearrange("(ko p) s -> p ko s", p=P)[:, :, sc * SC:(sc + 1) * SC]
                )
                for qk, w_sb in enumerate((wq_sb, wk_sb)):
                    for ft in range(2):
                        ps = psum.tile([P, SC], F32, tag="mm_ps", name="mm_ps")
                        for ko in range(KO):
                            nc.tensor.matmul(
                                ps[:],
                                _mm(w_sb[:, ko, ft * P:(ft + 1) * P]),
                                _mm(xt[:, ko, :]),
                                start=(ko == 0),
                                stop=(ko == KO - 1),
                            )
                        # PSUM -> SBUF eviction with per-feature bias (DVE
                        # tensor_scalar: scalar operand is per-partition).
                        bcol = 2 * qk + ft
                        if qk == 1:
                            nc.vector.tensor_scalar_add(
                                kT_sb[ft][:, sc * SC:(sc + 1) * SC],
                                ps[:],
                                bqk_sb[:, bcol:bcol + 1],
                            )
                        else:
                            for hr in range(2):
                                rr = slice(hr * HD, (hr + 1) * HD)
                                nc.vector.tensor_scalar_add(
                                    qT_sb[2 * ft + hr][rr, sc * SC:(sc + 1) * SC],
                                    ps[rr, :],
                                    bqk_sb[rr, bcol:bcol + 1],
                                )
                for st in range(SC // P):
                    g_s = sc * (SC // P) + st
                    ps = psum.tile([P, SC], F32, tag="mm_ps", name="mm_ps")[:, :FG]
                    for ko in range(KO):
                        nc.tensor.matmul(
                            ps[:],
                            _mm(xt[:, ko, st * P:(st + 1) * P]),
                            _mm(wv_sb[:, ko, :]),
                            start=(ko == 0),
                            stop=(ko == KO - 1),
                        )
                    nc.vector.memset(v_sb[g_s][:, :, HD:], 0.0)
                    nc.vector.memset(v_sb[g_s][:, :, HD], 1.0)
                    for h in range(HG):
                        nc.vector.tensor_tensor(
                            v_sb[g_s][:, h, 0:HD],
                            ps[:, h * HD:(h + 1) * HD],
                            bv_sb[:, h * HD:(h + 1) * HD],
                            mybir.AluOpType.add,
                        )

                # ---- causal attention for this chunk (S^T orientation) ----
                for h in range(HG):
                    hh = h // 2
                    n_j = (sc + 1) * (SC // P)
                    pv = psum.tile([P, SC], F32, tag="pv")
                    for jt in range(n_j):
                        o = jt - 4 * sc
                        off = max(0, 128 * o)  # diagonal blocks: skip i < j
                        sp = psum.tile([P, SC], F32, tag="score", bufs=3)
                        nc.tensor.matmul(
                            sp[:, off:],
                            _mm(kT_sb[hh][:, jt * P:(jt + 1) * P]),
                            _mm(qT_sb[h][:, sc * SC + off:(sc + 1) * SC]),
                            start=True,
                            stop=True,
                        )
                        pt = pt_pool.tile([P, SC], MM_DT, tag="pt")
                        # exp(scores / sqrt(HD)); scores are bounded, no max
                        nc.scalar.activation(
                            pt[:, off:], sp[:, off:],
                            mybir.ActivationFunctionType.Exp,
                            scale=1.0 / float(np.sqrt(HD)),
                        )
                        if o >= 0:
                            # in-band causal mask on the 128-wide diagonal
                            nc.vector.tensor_tensor(
                                pt[:, off:], pt[:, off:],
                                maskw_sb[:, 384:384 + SC - off],
                                mybir.AluOpType.mult,
                            )
                        nc.tensor.matmul(
                            pv[:, off:],
                            _mm(v_sb[jt][:, h, :]),
                            _mm(pt[:, off:]),
                            start=(jt == 0),
                            stop=(jt == n_j - 1),
                        )
                    lrow = small.tile([1, SC], F32, tag="lrow")
                    nc.vector.tensor_copy(lrow[:], pv[HD:HD + 1, :])
                    rec = small.tile([1, SC], F32, tag="rec")
                    nc.vector.reciprocal_approx_fast(rec[:], lrow[:])
                    rec_r = small.tile([1, SC], F32R, tag="rec_r")
                    nc.vector.tensor_copy(rec_r[:], rec[:])
                    rb = psum.tile([P, SC], F32, tag="rb", bufs=1, name="rb")
                    nc.tensor.matmul(rb[:], ones128[:], rec_r[:],
                                     start=True, stop=True)
                    rbs = small.tile([P, SC], F32, tag="rbs")
                    nc.vector.tensor_copy(rbs[:], rb[:])
                    nc.vector.tensor_tensor(
                        aT_sb[h][:, sc * SC:(sc + 1) * SC],
                        pv[:],
                        rbs[:],
                        mybir.AluOpType.mult,
                    )

                # ---- AllGather aT for this chunk across the 4-core group ----
                for h in range(HG):
                    nc.gpsimd.dma_start(
                        aT_loc[sc][h * HD:(h + 1) * HD, :],
                        aT_sb[h][0:HD, sc * SC:(sc + 1) * SC],
                    )
                nc.gpsimd.collective_compute(
                    "AllGather",
                    mybir.AluOpType.bypass,
                    ins=[aT_loc[sc][:].opt()],
                    outs=[aT_full[sc][:].opt()],
                    replica_groups=REPLICA_GROUPS,
                )

                # ---- output projection for this chunk (column slice) ----
                aTf = aTf_pool.tile([P, KO, SC], MM_DT, tag="aTf")
                nc.sync.dma_start(
                    aTf[:], aT_full[sc].rearrange("(ko p) s -> p ko s", p=P)
                )
                for st in range(SC // P):
                    ps = psum.tile([P, SC], F32, tag="mm_ps", name="mm_ps")[:, :FG]
                    for ko in range(KO):
                        nc.tensor.matmul(
                            ps[:],
                            _mm(aTf[:, ko, st * P:(st + 1) * P]),
                            _mm(wp_sb[:, ko, :]),
                            start=(ko == 0),
                            stop=(ko == KO - 1),
                        )
                    ot = out_pool.tile([P, FG], F32, tag="ot")
                    nc.vector.tensor_tensor(
                        ot[:], ps[:], bp_sb[:],
                        mybir.AluOpType.add,
                    )
                    nc.sync.dma_start(
                        out[sc * SC + st * P: sc * SC + (st + 1) * P, :], ot[:]
                    )
    return nc


_NC_CACHE = None


def _get_nc():
    global _NC_CACHE
    if _NC_CACHE is None:
        nc = bacc.Bacc("TRN2", target_bir_lowering=False, debug=False,
                       num_devices=N_CORES)
        build(nc)
        nc.compile()
        _NC_CACHE = nc
    return _NC_CACHE


def make_in_maps(x, c_attn_w, c_attn_b, c_proj_w, c_proj_b):
    x = np.asarray(x, dtype=np.float32)
    c_attn_w = np.asarray(c_attn_w, dtype=np.float32)
    c_attn_b = np.asarray(c_attn_b, dtype=np.float32)
    c_proj_w = np.asarray(c_proj_w, dtype=np.float32)
    c_proj_b = np.asarray(c_proj_b, dtype=np.float32)

    bf16 = ml_dtypes.bfloat16
    r = np.arange(P)[:, None]
    xcol = np.arange(896)[None, :]
    maskw = (xcol >= r + 384).astype(np.float32)

    in_maps = []
    for c in range(N_CORES):
        b, g = divmod(c, NG)
        fsl = slice(g * FG, (g + 1) * FG)
        bq = c_attn_b[0 * NX:1 * NX][fsl]
        bk = c_attn_b[1 * NX:2 * NX][fsl]
        in_maps.append({
            "xT": np.ascontiguousarray(x[b].T).astype(bf16),
            "wq": np.ascontiguousarray(c_attn_w[:, 0 * NX:1 * NX][:, fsl]).astype(bf16),
            "wk": np.ascontiguousarray(c_attn_w[:, 1 * NX:2 * NX][:, fsl]).astype(bf16),
            "wv": np.ascontiguousarray(c_attn_w[:, 2 * NX:3 * NX][:, fsl]).astype(bf16),
            "bqk": np.stack([bq[0:P], bq[P:2 * P], bk[0:P], bk[P:2 * P]], axis=1)
                     .astype(np.float32).copy(),
            "bv": np.repeat(c_attn_b[2 * NX:3 * NX][fsl][None, :], P, axis=0).copy(),
            "wp": np.ascontiguousarray(c_proj_w[:, fsl]).astype(bf16),
            "bp": np.repeat(c_proj_b[fsl][None, :], P, axis=0).copy(),
            "maskw": maskw.astype(bf16),
            "onesd": np.ones((1, P), dtype=np.float32),
        })
    return in_maps


def assemble(results):
    """[core]{'out': [S, FG]} -> [B, S, NX] by pure concatenation."""
    full = np.empty((B, S, NX), dtype=np.float32)
    for c in range(N_CORES):
        b, g = divmod(c, NG)
        full[b, :, g * FG:(g + 1) * FG] = results[c]["out"]
    return full


def kernel(x, c_attn_w, c_attn_b, c_proj_w, c_proj_b):
    nc = _get_nc()
    in_maps = make_in_maps(x, c_attn_w, c_attn_b, c_proj_w, c_proj_b)
    res = run_bass_kernel_spmd(nc, in_maps, core_ids=list(range(N_CORES)))
    return assemble(res.results)


# revision 18
# speedup vs baseline: 1.0512x; 1.0512x over previous
"""Distributed causal multi-head attention for 8 TRN2 NeuronCores.

Problem: y = (softmax(mask(Q K^T / sqrt(d))) V) @ c_proj_w + c_proj_b with
Q,K,V = split(x @ c_attn_w + c_attn_b), shapes B=2, S=2048, NX=1024, NH=16,
HD=64.

Sharding: core c = (b, g) with b = c // 4, g = c % 4 — data parallel over the
batch, tensor parallel over 4 head-groups of 4 heads. Each core:
  1. computes qT/kT ([d, s] layout) and v ([s, d] layout) for its 4 heads from
     a host-pretransposed x[b]^T, so no on-device transposes are ever needed;
  2. runs causal attention in the "S^T" orientation: scores come out of the PE
     as [j, i] tiles, exp() is fused into the PSUM->SBUF copy on the scalar
     engine (no max-subtraction — scores are bounded), and the softmax
     denominator falls out of the PV matmul for free via a ones-column
     appended to V;
  3. AllGathers aT = (attention output)^T across its 4-core group and computes
     a 256-wide column slice of the output projection.
The host wrapper only slices/transposes inputs and concatenates outputs.
"""

import ml_dtypes
import numpy as np

import concourse.bass as bass
import concourse.mybir as mybir
from concourse import bacc, tile
from concourse.bass_utils import run_bass_kernel_spmd

B, S, NX, NH, HD = 2, 2048, 1024, 16, 64
NG = 4              # head-groups == cores per batch entry
HG = NH // NG       # heads per core
FG = HG * HD        # local feature width (256)
P = 128
SC = 512            # sequence chunk width
NSC = S // SC       # 4 chunks
KO = NX // P        # 8 contraction tiles
N_CORES = 8

F32 = mybir.dt.float32

# Compute dtype for PE matmuls. bf16 runs the PE at full rate with
# hardware fast-weight-load; fp32r streams rows at full rate but pays a
# ~512-cycle self-contained weight load per matmul (LDW cannot be split
# or cached for fp32/fp32r), an intrinsic ~2x overhead at N=512.
MM_DT = mybir.dt.bfloat16
F32R = mybir.dt.float32r

REPLICA_GROUPS = [[0, 1, 2, 3], [4, 5, 6, 7]]


def _mm(ap):
    """Matmul operands are already typed as MM_DT."""
    return ap


def build(nc: bass.Bass):
    xT = nc.declare_dram_parameter("xT", [NX, S], MM_DT, isOutput=False)
    wq = nc.declare_dram_parameter("wq", [NX, FG], MM_DT, isOutput=False)
    wk = nc.declare_dram_parameter("wk", [NX, FG], MM_DT, isOutput=False)
    wv = nc.declare_dram_parameter("wv", [NX, FG], MM_DT, isOutput=False)
    bqk = nc.declare_dram_parameter("bqk", [P, 4], F32, isOutput=False)
    bv = nc.declare_dram_parameter("bv", [P, FG], MM_DT, isOutput=False)
    wp = nc.declare_dram_parameter("wp", [NX, FG], MM_DT, isOutput=False)
    bp = nc.declare_dram_parameter("bp", [P, FG], F32, isOutput=False)
    maskw = nc.declare_dram_parameter("maskw", [P, 896], MM_DT, isOutput=False)
    onesd = nc.declare_dram_parameter("onesd", [1, P], F32R, isOutput=False)
    out = nc.declare_dram_parameter("out", [S, FG], F32, isOutput=True)

    # Per-chunk collective bounce buffers (collectives can't touch kernel I/O).
    ag_warm_in = nc.dram_tensor("ag_warm_in", [4, 128], MM_DT)
    ag_warm_out = nc.dram_tensor("ag_warm_out", [16, 128], MM_DT)
    aT_loc = [nc.dram_tensor(f"aT_loc{c}", [FG, SC], MM_DT) for c in range(NSC)]
    aT_full = [
        nc.dram_tensor(f"aT_full{c}", [NG * FG, SC], MM_DT) for c in range(NSC)
    ]

    with tile.TileContext(nc) as tc:
        nc_lp = nc.allow_low_precision(reason="float32r PE compute path")
        nc_lp.__enter__()
        with (
            tc.tile_pool(name="consts", bufs=1) as consts,
            tc.tile_pool(name="persist", bufs=1) as persist,
            tc.tile_pool(name="xt", bufs=4) as xt_pool,
            tc.tile_pool(name="pt", bufs=8) as pt_pool,
            tc.tile_pool(name="aTf", bufs=2) as aTf_pool,
            tc.tile_pool(name="outs", bufs=3) as out_pool,
            tc.tile_pool(name="small", bufs=4) as small,
            tc.tile_pool(name="psum", bufs=2, space="PSUM") as psum,
        ):
            # ---- load weights / constants ----
            wq_sb = consts.tile([P, KO, FG], MM_DT)
            wk_sb = consts.tile([P, KO, FG], MM_DT)
            wv_sb = consts.tile([P, KO, FG], MM_DT)
            wp_sb = consts.tile([P, KO, FG], MM_DT)
            bqk_sb = consts.tile([P, 4], F32)
            bv_sb = consts.tile([P, FG], MM_DT)
            bp_sb = consts.tile([P, FG], F32)
            maskw_sb = consts.tile([P, 896], MM_DT)
            ones128 = consts.tile([1, P], F32R)
            nc.sync.dma_start(wq_sb[:], wq.rearrange("(ko p) f -> p ko f", p=P))
            nc.sync.dma_start(bqk_sb[:], bqk[:])
            nc.sync.dma_start(wk_sb[:], wk.rearrange("(ko p) f -> p ko f", p=P))
            nc.sync.dma_start(wv_sb[:], wv.rearrange("(ko p) f -> p ko f", p=P))
            nc.sync.dma_start(bv_sb[:], bv[:])
            nc.gpsimd.dma_start(maskw_sb[:], maskw[:])
            nc.gpsimd.dma_start(ones128[:], onesd[:])
            nc.gpsimd.dma_start(wp_sb[:], wp.rearrange("(ko p) f -> p ko f", p=P))
            nc.gpsimd.dma_start(bp_sb[:], bp[:])

            # ---- persistent activation tiles ----
            # kT: [d, s] packed — tile hh holds heads (2hh, 2hh+1) on
            # partition halves; it is the scores lhsT ([128,128] weight
            # loads keep the PE fast-weight-load path).
            # qT: one zero-padded [128, s] tile per head, data on the same
            # partition half as in kT, zeros elsewhere — the zeros select
            # the head out of the packed kT during the scores matmul.
            # v: [s, d] per 128-row tile, a ones column at col 64 (softmax
            # denominator) and zero padding out to 128 columns so the PV
            # lhsT is a full [128,128] block.
            # aT: per-head [128, s]; only rows 0:64 are meaningful.
            qT_sb = [persist.tile([P, S], MM_DT, name=f"qT{h}") for h in range(HG)]
            kT_sb = [persist.tile([P, S], MM_DT, name=f"kT{hh}") for hh in range(2)]
            v_sb = [persist.tile([P, HG, P], MM_DT, name=f"v{st}") for st in range(S // P)]
            aT_sb = [persist.tile([P, S], MM_DT, name=f"aT{h}") for h in range(HG)]
            for h in range(HG):
                pad0 = (1 - h % 2) * HD
                nc.vector.memset(qT_sb[h][pad0:pad0 + HD, :], 0.0)

            nc.gpsimd.collective_compute(
                "AllGather",
                mybir.AluOpType.bypass,
                ins=[ag_warm_in[:].opt()],
                outs=[ag_warm_out[:].opt()],
                replica_groups=REPLICA_GROUPS,
            )

            # ===== per-chunk pipeline: QKV -> attention -> AllGather -> proj
            # Ascending order: attention for chunk sc only needs K/V of
            # chunks <= sc, so QKV(sc+1) overlaps attention(sc) and the
            # per-chunk AllGathers spread across the whole kernel.
            xts = []
            for sc in range(NSC):
                xt = xt_pool.tile([P, KO, SC], MM_DT, tag="xt", name=f"xt{sc}")
                nc.sync.dma_start(
                    xt[:], xT.rearrange("(ko p) s -> p ko s", p=P)[:, :, sc * SC:(sc + 1) * SC]
                )
                xts.append(xt)

            for sc in range(NSC):
                # ---- QKV for this chunk ----
                xt = xts[sc]
                for qk, w_sb in enumerate((wq_sb, wk_sb)):
                    for ft in range(2):
                        ps = psum.tile([P, SC], F32, tag="mm_ps", name="mm_ps")
                        for ko in range(KO):
                            nc.tensor.matmul(
                                ps[:],
                                _mm(w_sb[:, ko, ft * P:(ft + 1) * P]),
                                _mm(xt[:, ko, :]),
                                start=(ko == 0),
                                stop=(ko == KO - 1),
                            )
                        # PSUM -> SBUF eviction with per-feature bias (DVE
                        # tensor_scalar: scalar operand is per-partition).
                        bcol = 2 * qk + ft
                        if qk == 1:
                            nc.vector.tensor_scalar_add(
                                kT_sb[ft][:, sc * SC:(sc + 1) * SC],
                                ps[:],
                                bqk_sb[:, bcol:bcol + 1],
                            )
                        else:
                            for hr in range(2):
                                rr = slice(hr * HD, (hr + 1) * HD)
                                nc.vector.tensor_scalar_add(
                                    qT_sb[2 * ft + hr][rr, sc * SC:(sc + 1) * SC],
                                    ps[rr, :],
                                    bqk_sb[rr, bcol:bcol + 1],
                                )
                for st in range(SC // P):
                    g_s = sc * (SC // P) + st
                    ps = psum.tile([P, SC], F32, tag="mm_ps", name="mm_ps")[:, :FG]
                    for ko in range(KO):
                        nc.tensor.matmul(
                            ps[:],
                            _mm(xt[:, ko, st * P:(st + 1) * P]),
                            _mm(wv_sb[:, ko, :]),
                            start=(ko == 0),
                            stop=(ko == KO - 1),
                        )
                    nc.vector.memset(v_sb[g_s][:, :, HD:], 0.0)
                    nc.vector.memset(v_sb[g_s][:, :, HD], 1.0)
                    for h in range(HG):
                        nc.vector.tensor_tensor(
                            v_sb[g_s][:, h, 0:HD],
                            ps[:, h * HD:(h + 1) * HD],
                            bv_sb[:, h * HD:(h + 1) * HD],
                            mybir.AluOpType.add,
                        )

                # ---- causal attention for this chunk (S^T orientation) ----
                for h in range(HG):
                    hh = h // 2
                    n_j = (sc + 1) * (SC // P)
                    pv = psum.tile([P, SC], F32, tag="pv")
                    for jt in range(n_j):
                        o = jt - 4 * sc
                        off = max(0, 128 * o)  # diagonal blocks: skip i < j
                        sp = psum.tile([P, SC], F32, tag="score", bufs=3)
                        nc.tensor.matmul(
                            sp[:, off:],
                            _mm(kT_sb[hh][:, jt * P:(jt + 1) * P]),
                            _mm(qT_sb[h][:, sc * SC + off:(sc + 1) * SC]),
                            start=True,
                            stop=True,
                        )
                        pt = pt_pool.tile([P, SC], MM_DT, tag="pt")
                        # exp(scores / sqrt(HD)); scores are bounded, no max
                        nc.scalar.activation(
                            pt[:, off:], sp[:, off:],
                            mybir.ActivationFunctionType.Exp,
                            scale=1.0 / float(np.sqrt(HD)),
                        )
                        if o >= 0:
                            # in-band causal mask on the 128-wide diagonal
                            nc.vector.tensor_tensor(
                                pt[:, off:], pt[:, off:],
                                maskw_sb[:, 384:384 + SC - off],
                                mybir.AluOpType.mult,
                            )
                        nc.tensor.matmul(
                            pv[:, off:],
                            _mm(v_sb[jt][:, h, :]),
                            _mm(pt[:, off:]),
                            start=(jt == 0),
                            stop=(jt == n_j - 1),
                        )
                    lrow = small.tile([1, SC], F32, tag="lrow")
                    nc.vector.tensor_copy(lrow[:], pv[HD:HD + 1, :])
                    rec = small.tile([1, SC], F32, tag="rec")
                    nc.vector.reciprocal_approx_fast(rec[:], lrow[:])
                    rec_r = small.tile([1, SC], F32R, tag="rec_r")
                    nc.vector.tensor_copy(rec_r[:], rec[:])
                    rb = psum.tile([P, SC], F32, tag="rb", bufs=1, name="rb")
                    nc.tensor.matmul(rb[:], ones128[:], rec_r[:],
                                     start=True, stop=True)
                    rbs = small.tile([P, SC], F32, tag="rbs")
                    nc.vector.tensor_copy(rbs[:], rb[:])
                    nc.vector.tensor_tensor(
                        aT_sb[h][:, sc * SC:(sc + 1) * SC],
                        pv[:],
                        rbs[:],
                        mybir.AluOpType.mult,
                    )

                # ---- AllGather aT for this chunk across the 4-core group ----
                for h in range(HG):
                    nc.gpsimd.dma_start(
                        aT_loc[sc][h * HD:(h + 1) * HD, :],
                        aT_sb[h][0:HD, sc * SC:(sc + 1) * SC],
                    )
                nc.gpsimd.collective_compute(
                    "AllGather",
                    mybir.AluOpType.bypass,
                    ins=[aT_loc[sc][:].opt()],
                    outs=[aT_full[sc][:].opt()],
                    replica_groups=REPLICA_GROUPS,
                )

                # ---- output projection for this chunk (column slice) ----
                aTf = aTf_pool.tile([P, KO, SC], MM_DT, tag="aTf")
                nc.sync.dma_start(
                    aTf[:], aT_full[sc].rearrange("(ko p) s -> p ko s", p=P)
                )
                for st in range(SC // P):
                    ps = psum.tile([P, SC], F32, tag="mm_ps", name="mm_ps")[:, :FG]
                    for ko in range(KO):
                        nc.tensor.matmul(
                            ps[:],
                            _mm(aTf[:, ko, st * P:(st + 1) * P]),
                            _mm(wp_sb[:, ko, :]),
                            start=(ko == 0),
                            stop=(ko == KO - 1),
                        )
                    ot = out_pool.tile([P, FG], F32, tag="ot")
                    nc.vector.tensor_tensor(
                        ot[:], ps[:], bp_sb[:],
                        mybir.AluOpType.add,
                    )
                    nc.sync.dma_start(
                        out[sc * SC + st * P: sc * SC + (st + 1) * P, :], ot[:]
                    )
    return nc


_NC_CACHE = None


def _get_nc():
    global _NC_CACHE
    if _NC_CACHE is None:
        nc = bacc.Bacc("TRN2", target_bir_lowering=False, debug=False,
                       num_devices=N_CORES)
        build(nc)
        nc.compile()
        _NC_CACHE = nc
    return _NC_CACHE


def make_in_maps(x, c_attn_w, c_attn_b, c_proj_w, c_proj_b):
    x = np.asarray(x, dtype=np.float32)
    c_attn_w = np.asarray(c_attn_w, dtype=np.float32)
    c_attn_b = np.asarray(c_attn_b, dtype=np.float32)
    c_proj_w = np.asarray(c_proj_w, dtype=np.float32)
    c_proj_b = np.asarray(c_proj_b, dtype=np.float32)

    bf16 = ml_dtypes.bfloat16
    r = np.arange(P)[:, None]
    xcol = np.arange(896)[None, :]
    maskw = (xcol >= r + 384).astype(np.float32)

    in_maps = []
    for c in range(N_CORES):
        b, g = divmod(c, NG)
        fsl = slice(g * FG, (g + 1) * FG)
        bq = c_attn_b[0 * NX:1 * NX][fsl]
        bk = c_attn_b[1 * NX:2 * NX][fsl]
        in_maps.append({
            "xT": np.ascontiguousarray(x[b].T).astype(bf16),
            "wq": np.ascontiguousarray(c_attn_w[:, 0 * NX:1 * NX][:, fsl]).astype(bf16),
            "wk": np.ascontiguousarray(c_attn_w[:, 1 * NX:2 * NX][:, fsl]).astype(bf16),
            "wv": np.ascontiguousarray(c_attn_w[:, 2 * NX:3 * NX][:, fsl]).astype(bf16),
            "bqk": np.stack([bq[0:P], bq[P:2 * P], bk[0:P], bk[P:2 * P]], axis=1)
                     .astype(np.float32).copy(),
            "bv": np.repeat(c_attn_b[2 * NX:3 * NX][fsl][None, :], P, axis=0).copy(),
            "wp": np.ascontiguousarray(c_proj_w[:, fsl]).astype(bf16),
            "bp": np.repeat(c_proj_b[fsl][None, :], P, axis=0).copy(),
            "maskw": maskw.astype(bf16),
            "onesd": np.ones((1, P), dtype=np.float32),
        })
    return in_maps


def assemble(results):
    """[core]{'out': [S, FG]} -> [B, S, NX] by pure concatenation."""
    full = np.empty((B, S, NX), dtype=np.float32)
    for c in range(N_CORES):
        b, g = divmod(c, NG)
        full[b, :, g * FG:(g + 1) * FG] = results[c]["out"]
    return full


def kernel(x, c_attn_w, c_attn_b, c_proj_w, c_proj_b):
    nc = _get_nc()
    in_maps = make_in_maps(x, c_attn_w, c_attn_b, c_proj_w, c_proj_b)
    res = run_bass_kernel_spmd(nc, in_maps, core_ids=list(range(N_CORES)))
    return assemble(res.results)


# revision 19
# speedup vs baseline: 1.5164x; 1.4426x over previous
"""Distributed causal multi-head attention for 8 TRN2 NeuronCores.

Problem: y = (softmax(mask(Q K^T / sqrt(d))) V) @ c_proj_w + c_proj_b with
Q,K,V = split(x @ c_attn_w + c_attn_b), shapes B=2, S=2048, NX=1024, NH=16,
HD=64.

Sharding: core c = (b, g) with b = c // 4, g = c % 4 — data parallel over the
batch, tensor parallel over 4 head-groups of 4 heads. Each core:
  1. computes qT/kT ([d, s] layout) and v ([s, d] layout) for its 4 heads from
     a host-pretransposed x[b]^T, so no on-device transposes are ever needed;
  2. runs causal attention in the "S^T" orientation: scores come out of the PE
     as [j, i] tiles, exp() is fused into the PSUM->SBUF copy on the scalar
     engine (no max-subtraction — scores are bounded), and the softmax
     denominator falls out of the PV matmul for free via a ones-column
     appended to V;
  3. AllGathers aT = (attention output)^T across its 4-core group and computes
     a 256-wide column slice of the output projection.
The host wrapper only slices/transposes inputs and concatenates outputs.
"""

import ml_dtypes
import numpy as np

import concourse.bass as bass
import concourse.mybir as mybir
from concourse import bacc, tile
from concourse.bass_utils import run_bass_kernel_spmd

B, S, NX, NH, HD = 2, 2048, 1024, 16, 64
NG = 4              # head-groups == cores per batch entry
HG = NH // NG       # heads per core
FG = HG * HD        # local feature width (256)
P = 128
SC = 512            # sequence chunk width
NSC = S // SC       # 4 chunks
KO = NX // P        # 8 contraction tiles
N_CORES = 8

F32 = mybir.dt.float32

# Compute dtype for PE matmuls. bf16 runs the PE at full rate with
# hardware fast-weight-load; fp32r streams rows at full rate but pays a
# ~512-cycle self-contained weight load per matmul (LDW cannot be split
# or cached for fp32/fp32r), an intrinsic ~2x overhead at N=512.
MM_DT = mybir.dt.bfloat16
F32R = mybir.dt.float32r

REPLICA_GROUPS = [[0, 1, 2, 3], [4, 5, 6, 7]]


def _mm(ap):
    """Matmul operands are already typed as MM_DT."""
    return ap


def build(nc: bass.Bass):
    xT = nc.declare_dram_parameter("xT", [NX, S], MM_DT, isOutput=False)
    wq = nc.declare_dram_parameter("wq", [NX, FG], MM_DT, isOutput=False)
    wk = nc.declare_dram_parameter("wk", [NX, FG], MM_DT, isOutput=False)
    wv = nc.declare_dram_parameter("wv", [NX, FG], MM_DT, isOutput=False)
    bqk = nc.declare_dram_parameter("bqk", [P, 4], F32, isOutput=False)
    bv = nc.declare_dram_parameter("bv", [P, FG], MM_DT, isOutput=False)
    wp = nc.declare_dram_parameter("wp", [NX, FG], MM_DT, isOutput=False)
    bp = nc.declare_dram_parameter("bp", [P, FG], F32, isOutput=False)
    maskw = nc.declare_dram_parameter("maskw", [P, 896], MM_DT, isOutput=False)
    onesd = nc.declare_dram_parameter("onesd", [1, P], F32R, isOutput=False)
    out = nc.declare_dram_parameter("out", [S, FG], F32, isOutput=True)

    # Per-chunk collective bounce buffers (collectives can't touch kernel I/O).
    ag_warm_in = nc.dram_tensor("ag_warm_in", [4, 128], MM_DT)
    ag_warm_out = nc.dram_tensor("ag_warm_out", [16, 128], MM_DT)
    aT_loc = [nc.dram_tensor(f"aT_loc{c}", [FG, SC], MM_DT) for c in range(NSC)]
    aT_full = [
        nc.dram_tensor(f"aT_full{c}", [NG * FG, SC], MM_DT) for c in range(NSC)
    ]

    with tile.TileContext(nc) as tc:
        nc_lp = nc.allow_low_precision(reason="float32r PE compute path")
        nc_lp.__enter__()
        with (
            tc.tile_pool(name="consts", bufs=1) as consts,
            tc.tile_pool(name="persist", bufs=1) as persist,
            tc.tile_pool(name="xt", bufs=4) as xt_pool,
            tc.tile_pool(name="pt", bufs=8) as pt_pool,
            tc.tile_pool(name="aTf", bufs=2) as aTf_pool,
            tc.tile_pool(name="outs", bufs=3) as out_pool,
            tc.tile_pool(name="small", bufs=4) as small,
            tc.tile_pool(name="psum", bufs=2, space="PSUM") as psum,
        ):
            # ---- load weights / constants ----
            wq_sb = consts.tile([P, KO, FG], MM_DT)
            wk_sb = consts.tile([P, KO, FG], MM_DT)
            wv_sb = consts.tile([P, KO, FG], MM_DT)
            wp_sb = consts.tile([P, KO, FG], MM_DT)
            bqk_sb = consts.tile([P, 4], F32)
            bv_sb = consts.tile([P, FG], MM_DT)
            bp_sb = consts.tile([P, FG], F32)
            maskw_sb = consts.tile([P, 896], MM_DT)
            ones128 = consts.tile([1, P], F32R)
            nc.sync.dma_start(wq_sb[:], wq.rearrange("(ko p) f -> p ko f", p=P))
            nc.sync.dma_start(bqk_sb[:], bqk[:])
            nc.sync.dma_start(wk_sb[:], wk.rearrange("(ko p) f -> p ko f", p=P))
            nc.sync.dma_start(wv_sb[:], wv.rearrange("(ko p) f -> p ko f", p=P))
            nc.sync.dma_start(bv_sb[:], bv[:])
            nc.gpsimd.dma_start(maskw_sb[:], maskw[:])
            nc.gpsimd.dma_start(ones128[:], onesd[:])
            nc.gpsimd.dma_start(wp_sb[:], wp.rearrange("(ko p) f -> p ko f", p=P))
            nc.gpsimd.dma_start(bp_sb[:], bp[:])

            # ---- persistent activation tiles ----
            # kT: [d, s] packed — tile hh holds heads (2hh, 2hh+1) on
            # partition halves; it is the scores lhsT ([128,128] weight
            # loads keep the PE fast-weight-load path).
            # qT: one zero-padded [128, s] tile per head, data on the same
            # partition half as in kT, zeros elsewhere — the zeros select
            # the head out of the packed kT during the scores matmul.
            # v: [s, d] per 128-row tile, a ones column at col 64 (softmax
            # denominator) and zero padding out to 128 columns so the PV
            # lhsT is a full [128,128] block.
            # aT: per-head [128, s]; only rows 0:64 are meaningful.
            qT_sb = [persist.tile([P, S], MM_DT, name=f"qT{h}") for h in range(HG)]
            kT_sb = [persist.tile([P, S], MM_DT, name=f"kT{hh}") for hh in range(2)]
            v_sb = [persist.tile([P, HG, P], MM_DT, name=f"v{st}") for st in range(S // P)]
            aT_sb = [persist.tile([P, S], MM_DT, name=f"aT{h}") for h in range(HG)]
            for h in range(HG):
                pad0 = (1 - h % 2) * HD
                nc.vector.memset(qT_sb[h][pad0:pad0 + HD, :], 0.0)

            nc.gpsimd.collective_compute(
                "AllGather",
                mybir.AluOpType.bypass,
                ins=[ag_warm_in[:].opt()],
                outs=[ag_warm_out[:].opt()],
                replica_groups=REPLICA_GROUPS,
            )

            # ===== per-chunk pipeline: QKV -> attention -> AllGather -> proj
            # Ascending order: attention for chunk sc only needs K/V of
            # chunks <= sc, so QKV(sc+1) overlaps attention(sc) and the
            # per-chunk AllGathers spread across the whole kernel.
            xts = []
            for sc in range(NSC):
                xt = xt_pool.tile([P, KO, SC], MM_DT, tag="xt", name=f"xt{sc}")
                nc.sync.dma_start(
                    xt[:], xT.rearrange("(ko p) s -> p ko s", p=P)[:, :, sc * SC:(sc + 1) * SC]
                )
                xts.append(xt)

            for sc in range(NSC):
                # ---- QKV for this chunk ----
                xt = xts[sc]
                for qk, w_sb in enumerate((wq_sb, wk_sb)):
                    for ft in range(2):
                        ps = psum.tile([P, SC], F32, tag="mm_ps", name="mm_ps")
                        for ko in range(KO):
                            nc.tensor.matmul(
                                ps[:],
                                _mm(w_sb[:, ko, ft * P:(ft + 1) * P]),
                                _mm(xt[:, ko, :]),
                                start=(ko == 0),
                                stop=(ko == KO - 1),
                            )
                        # PSUM -> SBUF eviction with per-feature bias (DVE
                        # tensor_scalar: scalar operand is per-partition).
                        bcol = 2 * qk + ft
                        if qk == 1:
                            nc.vector.tensor_scalar_add(
                                kT_sb[ft][:, sc * SC:(sc + 1) * SC],
                                ps[:],
                                bqk_sb[:, bcol:bcol + 1],
                            )
                        else:
                            for hr in range(2):
                                rr = slice(hr * HD, (hr + 1) * HD)
                                nc.vector.tensor_scalar_add(
                                    qT_sb[2 * ft + hr][rr, sc * SC:(sc + 1) * SC],
                                    ps[rr, :],
                                    bqk_sb[rr, bcol:bcol + 1],
                                )
                for st in range(SC // P):
                    g_s = sc * (SC // P) + st
                    ps = psum.tile([P, SC], F32, tag="mm_ps", name="mm_ps")[:, :FG]
                    for ko in range(KO):
                        nc.tensor.matmul(
                            ps[:],
                            _mm(xt[:, ko, st * P:(st + 1) * P]),
                            _mm(wv_sb[:, ko, :]),
                            start=(ko == 0),
                            stop=(ko == KO - 1),
                        )
                    nc.vector.memset(v_sb[g_s][:, :, HD:], 0.0)
                    nc.vector.memset(v_sb[g_s][:, :, HD], 1.0)
                    for h in range(HG):
                        nc.vector.tensor_tensor(
                            v_sb[g_s][:, h, 0:HD],
                            ps[:, h * HD:(h + 1) * HD],
                            bv_sb[:, h * HD:(h + 1) * HD],
                            mybir.AluOpType.add,
                        )

                # ---- causal attention for this chunk (S^T orientation) ----
                for h in range(HG):
                    hh = h // 2
                    n_j = (sc + 1) * (SC // P)
                    pv = psum.tile([P, SC], F32, tag="pv")
                    for jt in range(n_j):
                        o = jt - 4 * sc
                        off = max(0, 128 * o)  # diagonal blocks: skip i < j
                        sp = psum.tile([P, SC], F32, tag="score", bufs=2)
                        nc.tensor.matmul(
                            sp[:, off:],
                            _mm(kT_sb[hh][:, jt * P:(jt + 1) * P]),
                            _mm(qT_sb[h][:, sc * SC + off:(sc + 1) * SC]),
                            start=True,
                            stop=True,
                        )
                        pt = pt_pool.tile([P, SC], MM_DT, tag="pt")
                        # exp(scores / sqrt(HD)); scores are bounded, no max
                        nc.scalar.activation(
                            pt[:, off:], sp[:, off:],
                            mybir.ActivationFunctionType.Exp,
                            scale=1.0 / float(np.sqrt(HD)),
                        )
                        if o >= 0:
                            # in-band causal mask on the 128-wide diagonal
                            nc.vector.tensor_tensor(
                                pt[:, off:], pt[:, off:],
                                maskw_sb[:, 384:384 + SC - off],
                                mybir.AluOpType.mult,
                            )
                        nc.tensor.matmul(
                            pv[:, off:],
                            _mm(v_sb[jt][:, h, :]),
                            _mm(pt[:, off:]),
                            start=(jt == 0),
                            stop=(jt == n_j - 1),
                        )
                    lrow = small.tile([1, SC], F32, tag="lrow")
                    nc.vector.tensor_copy(lrow[:], pv[HD:HD + 1, :])
                    rec = small.tile([1, SC], F32, tag="rec")
                    nc.vector.reciprocal_approx_fast(rec[:], lrow[:])
                    rec_r = small.tile([1, SC], F32R, tag="rec_r")
                    nc.vector.tensor_copy(rec_r[:], rec[:])
                    rb = psum.tile([P, SC], F32, tag="rb", bufs=1, name="rb")
                    nc.tensor.matmul(rb[:], ones128[:], rec_r[:],
                                     start=True, stop=True)
                    rbs = small.tile([P, SC], F32, tag="rbs")
                    nc.vector.tensor_copy(rbs[:], rb[:])
                    nc.vector.tensor_tensor(
                        aT_sb[h][:, sc * SC:(sc + 1) * SC],
                        pv[:],
                        rbs[:],
                        mybir.AluOpType.mult,
                    )

                # ---- AllGather aT for this chunk across the 4-core group ----
                for h in range(HG):
                    nc.gpsimd.dma_start(
                        aT_loc[sc][h * HD:(h + 1) * HD, :],
                        aT_sb[h][0:HD, sc * SC:(sc + 1) * SC],
                    )
                nc.gpsimd.collective_compute(
                    "AllGather",
                    mybir.AluOpType.bypass,
                    ins=[aT_loc[sc][:].opt()],
                    outs=[aT_full[sc][:].opt()],
                    replica_groups=REPLICA_GROUPS,
                )

                # ---- output projection for this chunk (column slice) ----
                aTf = aTf_pool.tile([P, KO, SC], MM_DT, tag="aTf")
                nc.sync.dma_start(
                    aTf[:], aT_full[sc].rearrange("(ko p) s -> p ko s", p=P)
                )
                for st in range(SC // P):
                    ps = psum.tile([P, SC], F32, tag="proj_ps", bufs=1, name="proj_ps")[:, :FG]
                    for ko in range(KO):
                        nc.tensor.matmul(
                            ps[:],
                            _mm(aTf[:, ko, st * P:(st + 1) * P]),
                            _mm(wp_sb[:, ko, :]),
                            start=(ko == 0),
                            stop=(ko == KO - 1),
                        )
                    ot = out_pool.tile([P, FG], F32, tag="ot")
                    nc.vector.tensor_tensor(
                        ot[:], ps[:], bp_sb[:],
                        mybir.AluOpType.add,
                    )
                    nc.sync.dma_start(
                        out[sc * SC + st * P: sc * SC + (st + 1) * P, :], ot[:]
                    )
    return nc


_NC_CACHE = None


def _get_nc():
    global _NC_CACHE
    if _NC_CACHE is None:
        nc = bacc.Bacc("TRN2", target_bir_lowering=False, debug=False,
                       num_devices=N_CORES)
        build(nc)
        nc.compile()
        _NC_CACHE = nc
    return _NC_CACHE


def make_in_maps(x, c_attn_w, c_attn_b, c_proj_w, c_proj_b):
    x = np.asarray(x, dtype=np.float32)
    c_attn_w = np.asarray(c_attn_w, dtype=np.float32)
    c_attn_b = np.asarray(c_attn_b, dtype=np.float32)
    c_proj_w = np.asarray(c_proj_w, dtype=np.float32)
    c_proj_b = np.asarray(c_proj_b, dtype=np.float32)

    bf16 = ml_dtypes.bfloat16
    r = np.arange(P)[:, None]
    xcol = np.arange(896)[None, :]
    maskw = (xcol >= r + 384).astype(np.float32)

    in_maps = []
    for c in range(N_CORES):
        b, g = divmod(c, NG)
        fsl = slice(g * FG, (g + 1) * FG)
        bq = c_attn_b[0 * NX:1 * NX][fsl]
        bk = c_attn_b[1 * NX:2 * NX][fsl]
        in_maps.append({
            "xT": np.ascontiguousarray(x[b].T).astype(bf16),
            "wq": np.ascontiguousarray(c_attn_w[:, 0 * NX:1 * NX][:, fsl]).astype(bf16),
            "wk": np.ascontiguousarray(c_attn_w[:, 1 * NX:2 * NX][:, fsl]).astype(bf16),
            "wv": np.ascontiguousarray(c_attn_w[:, 2 * NX:3 * NX][:, fsl]).astype(bf16),
            "bqk": np.stack([bq[0:P], bq[P:2 * P], bk[0:P], bk[P:2 * P]], axis=1)
                     .astype(np.float32).copy(),
            "bv": np.repeat(c_attn_b[2 * NX:3 * NX][fsl][None, :], P, axis=0).copy(),
            "wp": np.ascontiguousarray(c_proj_w[:, fsl]).astype(bf16),
            "bp": np.repeat(c_proj_b[fsl][None, :], P, axis=0).copy(),
            "maskw": maskw.astype(bf16),
            "onesd": np.ones((1, P), dtype=np.float32),
        })
    return in_maps


def assemble(results):
    """[core]{'out': [S, FG]} -> [B, S, NX] by pure concatenation."""
    full = np.empty((B, S, NX), dtype=np.float32)
    for c in range(N_CORES):
        b, g = divmod(c, NG)
        full[b, :, g * FG:(g + 1) * FG] = results[c]["out"]
    return full


def kernel(x, c_attn_w, c_attn_b, c_proj_w, c_proj_b):
    nc = _get_nc()
    in_maps = make_in_maps(x, c_attn_w, c_attn_b, c_proj_w, c_proj_b)
    res = run_bass_kernel_spmd(nc, in_maps, core_ids=list(range(N_CORES)))
    return assemble(res.results)
